# revision 1
# baseline (speedup 1.0000x reference)
"""Neighbourhood attention block (7x7 clamped window) on 8 Trainium2 cores.

Sharding: (batch, head-pair) tensor parallel. Core c handles batch b = c//4
and heads (2*(c%4), 2*(c%4)+1). Each core computes q/k/v projections for its
two heads, neighbourhood attention, and a partial output projection
y_partial = ao @ w_out_slice^T. Host sums the 4 partials per batch.

Attention layout: scoresT [key, query] tiles so PV needs no transposes.
Keys are chunked 2 image rows (128 tokens) per chunk; each chunk is matched
against the 8 query rows that can see it (512 queries, one N=512 matmul).
Masking is multiplicative 0/1 after exp (bf16), so invalid keys contribute 0
to both PV numerator and the denominator (a ones-column appended to V).
"""
import os
import numpy as np
import ml_dtypes
from contextlib import ExitStack

_PHASES = os.environ.get("KERNEL_PHASES", "123")  # debug bisect knob

import concourse.bass as bass
import concourse.bacc as bacc
import concourse.tile as tile
import concourse.mybir as mybir
from concourse.bass_utils import run_bass_kernel_spmd
from concourse.masks import make_identity

F32 = mybir.dt.float32
F32R = mybir.dt.float32r
BF16 = mybir.dt.bfloat16

B, H, W, D = 2, 64, 64, 512
DH, NH = 64, 8
S = H * W              # 4096 tokens per batch
KER = 7
SCALE = DH ** -0.5     # 0.125
NCORES = 8

# ---------------------------------------------------------------- geometry

def _sh(r):            # clamped window start (rows); same formula for cols
    return min(max(r - KER // 2, 0), H - KER)


def _chunks_of_row(r):  # key chunks (2 rows each) seen by query row r
    s = _sh(r)
    return list(range(s // 2, (s + KER + 1) // 2))


def _build_plan():
    """Tiles: scoresT [128 keys of chunk c, qw queries at q0]. Groups: PV
    accumulations [65, qw] covering disjoint query ranges."""
    tiles = []          # dict(c, q0, qw)
    for c in range(32):
        q0r = min(max(2 * c - 3, 0), 56)
        tiles.append(dict(c=c, q0=q0r * 64, qw=512))
    for c in (2, 3):        # query rows 0..2 miss these chunks' main windows
        tiles.append(dict(c=c, q0=0, qw=192))
    for c in (28, 29):      # query rows 61..63
        tiles.append(dict(c=c, q0=61 * 64, qw=192))

    # rows covered by each tile, for resolution
    def covers(t, r):
        return t["q0"] <= r * 64 and (r + 1) * 64 <= t["q0"] + t["qw"]

    groups = []         # dict(q0, qw, rows)
    groups.append(dict(rows=[0]))
    groups.append(dict(rows=[1, 2]))
    for k in range(14):
        r0 = 4 * k + 3
        groups.append(dict(rows=[r0, r0 + 1, r0 + 2, r0 + 3]))
    groups.append(dict(rows=[59, 60, 61, 62]))
    groups.append(dict(rows=[63]))

    for g in groups:
        rows = g["rows"]
        g["q0"] = rows[0] * 64
        g["qw"] = len(rows) * 64
        # chunk -> contiguous row subrange of this group needing it
        chunk_rows = {}
        for r in rows:
            for c in _chunks_of_row(r):
                a, b = chunk_rows.get(c, (r, r))
                chunk_rows[c] = (min(a, r), max(b, r))
        mms = []        # (c, row_a, row_b_inclusive, width)
        for c, (ra, rb) in sorted(chunk_rows.items()):
            mms.append((c, ra, rb, (rb - ra + 1) * 64))
        mms.sort(key=lambda m: -m[3])   # widest (full-group) first for start=True
        assert mms[0][3] == g["qw"], (g, mms)
        # resolve each (c, row range) to probs-tile segments
        segs = []       # (c, tile_i, tile_off, out_off, length)
        for c, ra, rb, _w in mms:
            r = ra
            while r <= rb:
                cand = [i for i, t in enumerate(tiles) if t["c"] == c and covers(t, r)]
                assert cand, (g, c, r)
                ti = cand[0]
                t = tiles[ti]
                # extend run while same tile covers
                r2 = r
                while r2 + 1 <= rb and covers(t, r2 + 1):
                    r2 += 1
                segs.append((c, ti, r * 64 - t["q0"], r * 64 - g["q0"],
                             (r2 - r + 1) * 64))
                r = r2 + 1
        g["segs"] = segs

    # sanity: every (query row, chunk) incidence consumed exactly once
    seen = set()
    for g in groups:
        for c, ti, toff, ooff, ln in g["segs"]:
            for r in range((g["q0"] + ooff) // 64, (g["q0"] + ooff + ln) // 64):
                key = (r, c)
                assert key not in seen, key
                seen.add(key)
    for r in range(H):
        for c in _chunks_of_row(r):
            assert (r, c) in seen, (r, c)

    # masks per tile (0/1), deduped
    starts = np.minimum(np.maximum(np.arange(H) - KER // 2, 0), H - KER)
    valid = (np.arange(H)[None, :] >= starts[:, None]) & \
            (np.arange(H)[None, :] < starts[:, None] + KER)   # [q pos, k pos]

    def tile_mask(t):
        ktok = t["c"] * 128 + np.arange(128)
        qtok = t["q0"] + np.arange(t["qw"])
        m = np.zeros((128, 512), np.float32)
        m[:, :t["qw"]] = (valid[qtok[None, :] // 64, ktok[:, None] // 64]
                          & valid[qtok[None, :] % 64, ktok[:, None] % 64])
        return m

    mask_list, mask_ids = [], {}
    for t in tiles:
        m = tile_mask(t)
        key = m.tobytes()
        if key not in mask_ids:
            mask_ids[key] = len(mask_list)
            mask_list.append(m)
        t["mask_id"] = mask_ids[key]
    return tiles, groups, np.stack(mask_list)


TILES, GROUPS, MASKS = _build_plan()
NMASK = len(MASKS)

# ---------------------------------------------------------------- device

_NC_CACHE = {}
TRACE = False          # set True (e.g. from test.py) to capture an NTFF profile
LAST_RESULTS = None    # BassKernelResults of the most recent kernel() call


def _build_module():
    nc = bacc.Bacc("TRN2", target_bir_lowering=False, debug=False,
                   num_devices=NCORES)
    xT_d = nc.dram_tensor("xT", [D, S], F32R, kind="ExternalInput")
    wq_d = nc.dram_tensor("wq", [D, 128], F32R, kind="ExternalInput")
    wk_d = nc.dram_tensor("wk", [D, 128], F32R, kind="ExternalInput")
    wv_d = nc.dram_tensor("wv", [D, 128], F32R, kind="ExternalInput")
    wo_d = nc.dram_tensor("wo", [128, 512], F32R, kind="ExternalInput")
    mk_d = nc.dram_tensor("masks", [NMASK, 128, 512], BF16, kind="ExternalInput")
    y_d = nc.dram_tensor("y", [S, D], F32, kind="ExternalOutput")

    with tile.TileContext(nc) as tc, ExitStack() as ctx:
        const = ctx.enter_context(tc.tile_pool(name="const", bufs=1))
        xT_t = const.tile([128, 4, S], F32R, tag="xT")
        nc.sync.dma_start(out=xT_t[:], in_=xT_d.ap().rearrange("(c p) t -> p c t", p=128))
        wq_t = const.tile([128, 4, 128], F32R, tag="wq")
        nc.sync.dma_start(out=wq_t[:], in_=wq_d.ap().rearrange("(c p) m -> p c m", p=128))
        wk_t = const.tile([128, 4, 128], F32R, tag="wk")
        nc.sync.dma_start(out=wk_t[:], in_=wk_d.ap().rearrange("(c p) m -> p c m", p=128))
        wv_t = const.tile([128, 4, 128], F32R, tag="wv")
        nc.sync.dma_start(out=wv_t[:], in_=wv_d.ap().rearrange("(c p) m -> p c m", p=128))
        wo_t = const.tile([128, 512], F32R, tag="wo")
        nc.sync.dma_start(out=wo_t[:], in_=wo_d[:, :])
        mk_t = const.tile([128, NMASK, 512], BF16, tag="mk")
        nc.sync.dma_start(out=mk_t[:], in_=mk_d.ap().rearrange("m p w -> p m w"))

        qT = const.tile([128, S], F32R, tag="qT")      # [2 heads x 64e, tok]
        kT = const.tile([128, S], F32R, tag="kT")
        vT = const.tile([128, S], F32, tag="vT")
        # V: [tok_in_chunk, chunk, 130]: cols 0:64 u0-e, 64 ones, 65:129 u1-e, 129 ones
        V = const.tile([128, 32, 130], BF16, tag="V")
        nc.gpsimd.memset(V[:], 1.0)
        aoT = const.tile([128, S], F32R, tag="aoT")
        ident = const.tile([128, 128], F32, tag="ident")
        make_identity(nc, ident[:])

        # ---- phase 1: projections (f32r, N=512)
        with tc.tile_pool(name="pps", bufs=3, space="PSUM") as pps:
            for w_t, dst in ((wq_t, qT), (wk_t, kT), (wv_t, vT)) if "1" in _PHASES else ():
                for nb in range(8):
                    acc = pps.tile([128, 512], F32, tag="acc")
                    for dc in range(4):
                        nc.tensor.matmul(acc[:], w_t[:, dc, :],
                                         xT_t[:, dc, nb * 512:(nb + 1) * 512],
                                         start=(dc == 0), stop=(dc == 3))
                    nc.vector.tensor_copy(dst[:, nb * 512:(nb + 1) * 512], acc[:])
            # V transpose: both units at once, [128, 128] per chunk
            for pci in range(16) if "1" in _PHASES else ():
                tp = pps.tile([128, 2, 128], F32, tag="tp")
                for s in range(2):
                    ci = pci * 2 + s
                    nc.tensor.transpose(tp[:, s, :],
                                        vT[:, ci * 128:(ci + 1) * 128], ident[:])
                nc.vector.tensor_copy(V[:, pci * 2:pci * 2 + 2, 0:64],
                                      tp[:, :, 0:64])
                nc.vector.tensor_copy(V[:, pci * 2:pci * 2 + 2, 65:129],
                                      tp[:, :, 64:128])

        # ---- phase 2: attention
        with tc.tile_pool(name="sps", bufs=3, space="PSUM") as sps, \
             tc.tile_pool(name="pvs", bufs=3, space="PSUM") as pvs, \
             tc.tile_pool(name="prp", bufs=12) as prp, \
             tc.tile_pool(name="aux", bufs=4) as aux:
            alt = 0
            for u in ((0, 1) if "2" in _PHASES else ()):
                ue = slice(u * 64, u * 64 + 64)
                emitted = {}
                def emit_tile(ti):
                    nonlocal alt
                    t = TILES[ti]
                    qw, c = t["qw"], t["c"]
                    sc = sps.tile([128, 512], F32, tag="sc")
                    nc.tensor.matmul(sc[:, :qw],
                                     kT[ue, c * 128:(c + 1) * 128],
                                     qT[ue, t["q0"]:t["q0"] + qw],
                                     start=True, stop=True)
                    pr = prp.tile([128, 512], BF16, tag="pr")
                    nc.scalar.activation(pr[:, :qw], sc[:, :qw],
                                         mybir.ActivationFunctionType.Exp,
                                         scale=SCALE)
                    eng = nc.vector if alt % 3 != 2 else nc.gpsimd
                    alt += 1
                    eng.tensor_mul(pr[:, :qw], pr[:, :qw],
                                   mk_t[:, t["mask_id"], :qw])
                    emitted[ti] = pr

                uv = slice(u * 65, u * 65 + 65)
                for g in GROUPS:
                    for _c, ti, _to, _oo, _ln in g["segs"]:
                        if ti not in emitted:
                            emit_tile(ti)
                    qw = g["qw"]
                    pv = pvs.tile([65, 256], F32, tag="pv")
                    nseg = len(g["segs"])
                    for si, (c, ti, toff, ooff, ln) in enumerate(g["segs"]):
                        nc.tensor.matmul(pv[:, ooff:ooff + ln],
                                         V[:, c, uv],
                                         emitted[ti][:, toff:toff + ln],
                                         start=(si == 0), stop=(si == nseg - 1))
                    recip = aux.tile([1, 256], F32, tag="recip")
                    nc.vector.reciprocal(recip[:, :qw], pv[64:65, :qw])
                    rb = aux.tile([64, 256], F32, tag="rb")
                    nc.gpsimd.partition_broadcast(rb[:, :qw], recip[:, :qw])
                    nc.vector.tensor_mul(aoT[ue, g["q0"]:g["q0"] + qw],
                                         pv[0:64, :qw], rb[:, :qw])

        # ---- phase 3: output projection (partial; host sums across cores)
        with tc.tile_pool(name="ops", bufs=3, space="PSUM") as ops, \
             tc.tile_pool(name="yvp", bufs=4) as yvp:
            for tcn in range(32) if "3" in _PHASES else ():
                acc = ops.tile([128, 512], F32, tag="oacc")
                nc.tensor.matmul(acc[:], aoT[:, tcn * 128:(tcn + 1) * 128],
                                 wo_t[:], start=True, stop=True)
                yv = yvp.tile([128, 512], F32, tag="yv")
                if tcn % 2 == 0:
                    nc.vector.tensor_copy(yv[:], acc[:])
                else:
                    nc.scalar.activation(yv[:], acc[:],
                                         mybir.ActivationFunctionType.Copy)
                nc.sync.dma_start(out=y_d[tcn * 128:(tcn + 1) * 128, :], in_=yv[:])
    nc.compile()
    return nc


def _get_module():
    if "nc" not in _NC_CACHE:
        _NC_CACHE["nc"] = _build_module()
    return _NC_CACHE["nc"]


# ---------------------------------------------------------------- host

def kernel(x, w_qkv, w_out):
    x = np.asarray(x, np.float32)
    w_qkv = np.asarray(w_qkv, np.float32)
    w_out = np.asarray(w_out, np.float32)
    nc = _get_module()

    masks_bf16 = MASKS.astype(ml_dtypes.bfloat16)
    xT = [np.ascontiguousarray(x[b].reshape(S, D).T) for b in range(B)]
    w_outT = np.ascontiguousarray(w_out.T)

    in_maps = []
    for c in range(NCORES):
        b, h0 = c // 4, 2 * (c % 4)
        f = h0 * 64
        in_maps.append({
            "xT": xT[b],
            "wq": np.ascontiguousarray(w_qkv[f:f + 128].T),
            "wk": np.ascontiguousarray(w_qkv[512 + f:512 + f + 128].T),
            "wv": np.ascontiguousarray(w_qkv[1024 + f:1024 + f + 128].T),
            "wo": np.ascontiguousarray(w_outT[f:f + 128]),
            "masks": masks_bf16,
        })
    res = run_bass_kernel_spmd(nc, in_maps, list(range(NCORES)), trace=TRACE)
    global LAST_RESULTS
    LAST_RESULTS = res
    y = np.zeros((B, S, D), np.float32)
    for c in range(NCORES):
        y[c // 4] += res.results[c]["y"]
    return y.reshape(B, H, W, D)



# revision 8
# speedup vs baseline: 3.1616x; 3.1616x over previous
"""Neighbourhood attention block (7x7 clamped window) on 8 Trainium2 cores.

Sharding: (batch, head-pair) tensor parallel. Core c handles batch b = c//4
and heads (2*(c%4), 2*(c%4)+1). Each core computes q/k/v projections for its
two heads, neighbourhood attention, and a partial output projection; host
sums the 4 bf16 partials per batch in fp32.

v2 layout: all matmul operands bf16 (FWL weight loads, halved DMA).
Scores stay in scoresT [key, query] tiles, two tiles paired per 2-bank PSUM
so one Exp activation covers 1024 columns. PV flips orientation: probs
slices are the stationary operand so the PV output is [query-partition,
channel], which makes the softmax denominator a [128,1] reciprocal plus a
per-partition tensor_scalar multiply. Queries are grouped in odd-aligned
2-row blocks (rows 2j-1, 2j): such a block's 7-row key window spans exactly
the 8 query rows covered by the existing 512-wide score tiles, so every
(block, chunk) PV matmul is a contiguous 128-column slice of one tile.
Each block then transposes its [q, ch] attention output on the PE and runs
its own output-projection matmul, streaming y out per block.
"""
import numpy as np
import ml_dtypes
from contextlib import ExitStack

import concourse.bass as bass
import concourse.bacc as bacc
import concourse.tile as tile
import concourse.mybir as mybir
from concourse.bass_utils import run_bass_kernel_spmd
from concourse.masks import make_identity

F32 = mybir.dt.float32
BF16 = mybir.dt.bfloat16

B, H, W, D = 2, 64, 64, 512
DH, NH = 64, 8
S = H * W              # 4096 tokens per batch
KER = 7
SCALE = DH ** -0.5     # 0.125
NCORES = 8

# u1 data sits at col 96 in the PV bank so its 4-byte span stays 8B-aligned
U1 = 96

# ---------------------------------------------------------------- geometry

def _sh(r):            # clamped window start (rows); same formula for cols
    return min(max(r - KER // 2, 0), H - KER)


def _chunks_of_row(r):  # key chunks (2 rows each) seen by query row r
    s = _sh(r)
    return list(range(s // 2, (s + KER + 1) // 2))


def _build_plan():
    """TILES: scoresT [128 keys of chunk c, qw queries at q0], paired (2i,
    2i+1) into one 2-bank psum + one exp. BLOCKS: odd-aligned 2-row query
    blocks; each (block, chunk) resolves to a contiguous 128-col slice of
    one tile."""
    tiles = []
    for c in range(32):
        q0r = min(max(2 * c - 3, 0), 56)
        tiles.append(dict(c=c, q0=q0r * 64, qw=512))
    for c in (2, 3):        # query rows 0..2 miss these chunks' main windows
        tiles.append(dict(c=c, q0=0, qw=192))
    for c in (28, 29):      # query rows 61..63
        tiles.append(dict(c=c, q0=61 * 64, qw=192))

    blocks = [dict(rows=[0])]
    for j in range(1, 32):
        blocks.append(dict(rows=[2 * j - 1, 2 * j]))
    blocks.append(dict(rows=[63]))

    seen = set()
    for blk in blocks:
        rows = blk["rows"]
        blk["q0"] = rows[0] * 64
        blk["qw"] = len(rows) * 64
        chunks = sorted({c for r in rows for c in _chunks_of_row(r)})
        segs = []       # (chunk, tile_i, tile_off)
        for c in chunks:
            cand = [i for i, t in enumerate(tiles)
                    if t["c"] == c and t["q0"] <= blk["q0"]
                    and blk["q0"] + blk["qw"] <= t["q0"] + t["qw"]]
            assert cand, (blk, c)
            segs.append((c, cand[0], blk["q0"] - tiles[cand[0]]["q0"]))
        blk["segs"] = segs
        for r in rows:
            for c in _chunks_of_row(r):
                assert (r, c) not in seen
                seen.add((r, c))
    for r in range(H):
        for c in _chunks_of_row(r):
            assert (r, c) in seen, (r, c)

    # masks per tile-pair (0/1), deduped: [128 keys, 2, 512]
    starts = np.minimum(np.maximum(np.arange(H) - KER // 2, 0), H - KER)
    valid = (np.arange(H)[None, :] >= starts[:, None]) & \
            (np.arange(H)[None, :] < starts[:, None] + KER)   # [q pos, k pos]

    def tile_mask(t):
        ktok = t["c"] * 128 + np.arange(128)
        qtok = t["q0"] + np.arange(t["qw"])
        m = np.zeros((128, 512), np.float32)
        m[:, :t["qw"]] = (valid[qtok[None, :] // 64, ktok[:, None] // 64]
                          & valid[qtok[None, :] % 64, ktok[:, None] % 64])
        return m

    mask_list, mask_ids = [], {}
    pair_mask_id = []
    for pi in range(len(tiles) // 2):
        m = np.stack([tile_mask(tiles[2 * pi]), tile_mask(tiles[2 * pi + 1])],
                     axis=1)          # [128, 2, 512]
        key = m.tobytes()
        if key not in mask_ids:
            mask_ids[key] = len(mask_list)
            mask_list.append(m)
        pair_mask_id.append(mask_ids[key])
    return tiles, blocks, pair_mask_id, np.stack(mask_list)


TILES, BLOCKS, PAIR_MASK_ID, MASKS = _build_plan()
NPM = len(MASKS)

# ---------------------------------------------------------------- device

_NC_CACHE = {}
TRACE = False          # set True (e.g. from test.py) to capture an NTFF profile
LAST_RESULTS = None    # BassKernelResults of the most recent kernel() call


def _build_module():
    nc = bacc.Bacc("TRN2", target_bir_lowering=False, debug=False,
                   num_devices=NCORES)
    xT_d = nc.dram_tensor("xT", [D, S], BF16, kind="ExternalInput")
    wq_d = nc.dram_tensor("wq", [D, 128], BF16, kind="ExternalInput")
    wk_d = nc.dram_tensor("wk", [D, 128], BF16, kind="ExternalInput")
    wv_d = nc.dram_tensor("wv", [D, 128], BF16, kind="ExternalInput")
    wo_d = nc.dram_tensor("wo", [128, 512], BF16, kind="ExternalInput")
    mk_d = nc.dram_tensor("masks", [NPM * 2, 128, 512], BF16, kind="ExternalInput")
    y_d = nc.dram_tensor("y", [S, D], BF16, kind="ExternalOutput")

    with tile.TileContext(nc) as tc, ExitStack() as ctx:
        const = ctx.enter_context(tc.tile_pool(name="const", bufs=1))
        xT_t = const.tile([128, 4, S], BF16, tag="xT")
        xr = xT_d.ap().rearrange("(c p) t -> p c t", p=128)
        for ts in range(8):     # split so projections start early
            sl = slice(ts * 512, (ts + 1) * 512)
            nc.sync.dma_start(out=xT_t[:, :, sl], in_=xr[:, :, sl])
        wq_t = const.tile([128, 4, 128], BF16, tag="wq")
        nc.sync.dma_start(out=wq_t[:], in_=wq_d.ap().rearrange("(c p) m -> p c m", p=128))
        wk_t = const.tile([128, 4, 128], BF16, tag="wk")
        nc.sync.dma_start(out=wk_t[:], in_=wk_d.ap().rearrange("(c p) m -> p c m", p=128))
        wv_t = const.tile([128, 4, 128], BF16, tag="wv")
        nc.sync.dma_start(out=wv_t[:], in_=wv_d.ap().rearrange("(c p) m -> p c m", p=128))
        wo_t = const.tile([128, 512], BF16, tag="wo")
        nc.sync.dma_start(out=wo_t[:], in_=wo_d[:, :])
        mk_t = const.tile([128, NPM * 2, 512], BF16, tag="mk")
        nc.sync.dma_start(out=mk_t[:],
                          in_=mk_d.ap().rearrange("m p w -> p m w"))

        qT = const.tile([128, S], BF16, tag="qT")      # [2 heads x 64e, tok]
        kT = const.tile([128, S], BF16, tag="kT")
        # V: [tok_in_chunk, chunk, 130]: cols 0:64 u0-e, 64 ones, 65:129 u1-e, 129 ones
        V = const.tile([128, 32, 130], BF16, tag="V")
        nc.gpsimd.memset(V[:], 1.0)
        ident = const.tile([128, 128], BF16, tag="ident")
        make_identity(nc, ident[:])

        # ---- phase 1: projections
        with tc.tile_pool(name="pps", bufs=3, space="PSUM") as pps:
            for w_t, dst in ((wq_t, qT), (wk_t, kT)):
                for nb in range(8):
                    acc = pps.tile([128, 512], F32, tag="acc")
                    for dc in range(4):
                        nc.tensor.matmul(acc[:], w_t[:, dc, :],
                                         xT_t[:, dc, nb * 512:(nb + 1) * 512],
                                         start=(dc == 0), stop=(dc == 3))
                    nc.vector.tensor_copy(dst[:, nb * 512:(nb + 1) * 512], acc[:])
            # V in [token, channel] layout directly: xT chunk stationary
            for vb in range(8):
                acc = pps.tile([128, 4, 128], F32, tag="vacc")
                for t4 in range(4):
                    tok0 = (vb * 4 + t4) * 128
                    for dc in range(4):
                        nc.tensor.matmul(acc[:, t4, :],
                                         xT_t[:, dc, tok0:tok0 + 128],
                                         wv_t[:, dc, :],
                                         start=(dc == 0), stop=(dc == 3))
                nc.vector.tensor_copy(V[:, vb * 4:(vb + 1) * 4, 0:64],
                                      acc[:, :, 0:64])
                nc.vector.tensor_copy(V[:, vb * 4:(vb + 1) * 4, 65:129],
                                      acc[:, :, 64:128])

        # ---- phase 2: attention + per-block output projection
        with tc.tile_pool(name="scp", bufs=2, space="PSUM") as scp, \
             tc.tile_pool(name="pvp", bufs=2, space="PSUM") as pvp, \
             tc.tile_pool(name="typ", bufs=1, space="PSUM") as typ, \
             tc.tile_pool(name="prp", bufs=5) as prp, \
             tc.tile_pool(name="aop", bufs=3) as aop, \
             tc.tile_pool(name="atp", bufs=3) as atp, \
             tc.tile_pool(name="rcp", bufs=3) as rcp, \
             tc.tile_pool(name="ysp", bufs=3) as ysp:
            emitted = {}
            alt = [0]

            def ensure_pair(u, pi):
                if (u, pi) in emitted:
                    return
                ue = slice(u * 64, u * 64 + 64)
                sc = scp.tile([128, 2, 512], F32, tag="sc")
                for s in (0, 1):
                    t = TILES[2 * pi + s]
                    qw, c = t["qw"], t["c"]
                    nc.tensor.matmul(sc[:, s, :qw],
                                     kT[ue, c * 128:(c + 1) * 128],
                                     qT[ue, t["q0"]:t["q0"] + qw],
                                     start=True, stop=True)
                pr = prp.tile([128, 2, 512], BF16, tag="pr")
                nc.scalar.activation(pr[:], sc[:],
                                     mybir.ActivationFunctionType.Exp,
                                     scale=SCALE)
                mid = PAIR_MASK_ID[pi]
                eng = nc.vector if alt[0] % 2 == 0 else nc.gpsimd
                alt[0] += 1
                eng.tensor_mul(pr[:], pr[:], mk_t[:, 2 * mid:2 * mid + 2, :])
                emitted[(u, pi)] = pr

            for blk in BLOCKS:
                qw, q0 = blk["qw"], blk["q0"]
                for u in (0, 1):
                    for c, ti, off in blk["segs"]:
                        ensure_pair(u, ti // 2)
                pv = pvp.tile([128, 512], F32, tag="pv")
                nseg = len(blk["segs"])
                # all u0 matmuls strictly before u1: the u1 group's start=True
                # clears the whole bank's has_written bits
                for u in (0, 1):
                    u0c = 0 if u == 0 else U1
                    for si, (c, ti, off) in enumerate(blk["segs"]):
                        pr = emitted[(u, ti // 2)]
                        nc.tensor.matmul(pv[:qw, u0c:u0c + 65],
                                         pr[:, ti % 2, off:off + qw],
                                         V[:, c, u * 65:u * 65 + 65],
                                         start=(si == 0), stop=(si == nseg - 1))
                rc = rcp.tile([128, 2], F32, tag="rc")
                nc.vector.reciprocal(rc[:qw, 0:1], pv[:qw, 64:65])
                nc.vector.reciprocal(rc[:qw, 1:2], pv[:qw, U1 + 64:U1 + 65])
                ao = aop.tile([128, 128], BF16, tag="ao")
                nc.vector.tensor_scalar_mul(ao[:qw, 0:64], pv[:qw, 0:64],
                                            rc[:qw, 0:1])
                nc.vector.tensor_scalar_mul(ao[:qw, 64:128], pv[:qw, U1:U1 + 64],
                                            rc[:qw, 1:2])
                tr = typ.tile([128, 1024], BF16, tag="tr")
                nc.tensor.transpose(tr[:, 0:qw], ao[:qw, :], ident[0:qw, 0:qw])
                at = atp.tile([128, 128], BF16, tag="at")
                nc.vector.tensor_copy(at[:, 0:qw], tr[:, 0:qw])
                yo = typ.tile([128, 512], F32, tag="yo")
                nc.tensor.matmul(yo[:qw, :], at[:, 0:qw], wo_t[:],
                                 start=True, stop=True)
                ys = ysp.tile([128, 512], BF16, tag="ys")
                if alt[0] % 2 == 0:
                    nc.vector.tensor_copy(ys[:qw, :], yo[:qw, :])
                else:
                    nc.scalar.activation(ys[:qw, :], yo[:qw, :],
                                         mybir.ActivationFunctionType.Copy)
                alt[0] += 1
                nc.sync.dma_start(out=y_d[q0:q0 + qw, :], in_=ys[:qw, :])
    nc.compile()
    return nc


def _get_module():
    if "nc" not in _NC_CACHE:
        _NC_CACHE["nc"] = _build_module()
    return _NC_CACHE["nc"]


# ---------------------------------------------------------------- host

def kernel(x, w_qkv, w_out):
    x = np.asarray(x, np.float32)
    w_qkv = np.asarray(w_qkv, np.float32)
    w_out = np.asarray(w_out, np.float32)
    nc = _get_module()

    bf = ml_dtypes.bfloat16
    # [NPM, 128, 2, 512] -> [NPM*2, 128, 512] with (pair, slot) flattened
    masks_bf16 = np.ascontiguousarray(
        MASKS.transpose(0, 2, 1, 3).reshape(NPM * 2, 128, 512)).astype(bf)
    xT = [np.ascontiguousarray(x[b].reshape(S, D).T).astype(bf) for b in range(B)]
    w_outT = np.ascontiguousarray(w_out.T).astype(bf)
    w_qkv = w_qkv.astype(bf)

    in_maps = []
    for c in range(NCORES):
        b, h0 = c // 4, 2 * (c % 4)
        f = h0 * 64
        in_maps.append({
            "xT": xT[b],
            "wq": np.ascontiguousarray(w_qkv[f:f + 128].T),
            "wk": np.ascontiguousarray(w_qkv[512 + f:512 + f + 128].T),
            "wv": np.ascontiguousarray(w_qkv[1024 + f:1024 + f + 128].T),
            "wo": w_outT[f:f + 128],
            "masks": masks_bf16,
        })
    res = run_bass_kernel_spmd(nc, in_maps, list(range(NCORES)), trace=TRACE)
    global LAST_RESULTS
    LAST_RESULTS = res
    y = np.zeros((B, S, D), np.float32)
    for c in range(NCORES):
        y[c // 4] += res.results[c]["y"].astype(np.float32)
    return y.reshape(B, H, W, D)


# revision 17
# speedup vs baseline: 3.4103x; 1.0787x over previous
"""Neighbourhood attention block (7x7 clamped window) on 8 Trainium2 cores.

Sharding: (batch, head-pair) tensor parallel. Core c handles batch b = c//4
and heads (2*(c%4), 2*(c%4)+1). Each core computes q/k/v projections for its
two heads, neighbourhood attention, and a partial output projection; host
sums the 4 bf16 partials per batch in fp32.

v2 layout: all matmul operands bf16 (FWL weight loads, halved DMA).
Scores stay in scoresT [key, query] tiles, two tiles paired per 2-bank PSUM
so one Exp activation covers 1024 columns. PV flips orientation: probs
slices are the stationary operand so the PV output is [query-partition,
channel], which makes the softmax denominator a [128,1] reciprocal plus a
per-partition tensor_scalar multiply. Queries are grouped in odd-aligned
2-row blocks (rows 2j-1, 2j): such a block's 7-row key window spans exactly
the 8 query rows covered by the existing 512-wide score tiles, so every
(block, chunk) PV matmul is a contiguous 128-column slice of one tile.
Each block then transposes its [q, ch] attention output on the PE and runs
its own output-projection matmul, streaming y out per block.
"""
import numpy as np
import ml_dtypes
from contextlib import ExitStack

import concourse.bass as bass
import concourse.bacc as bacc
import concourse.tile as tile
import concourse.mybir as mybir
from concourse.bass_utils import run_bass_kernel_spmd
from concourse.masks import make_identity

F32 = mybir.dt.float32
BF16 = mybir.dt.bfloat16

B, H, W, D = 2, 64, 64, 512
DH, NH = 64, 8
S = H * W              # 4096 tokens per batch
KER = 7
SCALE = DH ** -0.5     # 0.125
NCORES = 8

# u1 data sits at col 96 in the PV bank so its 4-byte span stays 8B-aligned
U1 = 96

# ---------------------------------------------------------------- geometry

def _sh(r):            # clamped window start (rows); same formula for cols
    return min(max(r - KER // 2, 0), H - KER)


def _chunks_of_row(r):  # key chunks (2 rows each) seen by query row r
    s = _sh(r)
    return list(range(s // 2, (s + KER + 1) // 2))


def _build_plan():
    """TILES: scoresT [128 keys of chunk c, qw queries at q0], paired (2i,
    2i+1) into one 2-bank psum + one exp. BLOCKS: odd-aligned 2-row query
    blocks; each (block, chunk) resolves to a contiguous 128-col slice of
    one tile."""
    tiles = []
    for c in range(32):
        q0r = min(max(2 * c - 3, 0), 56)
        tiles.append(dict(c=c, q0=q0r * 64, qw=512))
    for c in (2, 3):        # query rows 0..2 miss these chunks' main windows
        tiles.append(dict(c=c, q0=0, qw=192))
    for c in (28, 29):      # query rows 61..63
        tiles.append(dict(c=c, q0=61 * 64, qw=192))

    blocks = [dict(rows=[0])]
    for j in range(1, 32):
        blocks.append(dict(rows=[2 * j - 1, 2 * j]))
    blocks.append(dict(rows=[63]))

    seen = set()
    for blk in blocks:
        rows = blk["rows"]
        blk["q0"] = rows[0] * 64
        blk["qw"] = len(rows) * 64
        chunks = sorted({c for r in rows for c in _chunks_of_row(r)})
        segs = []       # (chunk, tile_i, tile_off)
        for c in chunks:
            cand = [i for i, t in enumerate(tiles)
                    if t["c"] == c and t["q0"] <= blk["q0"]
                    and blk["q0"] + blk["qw"] <= t["q0"] + t["qw"]]
            assert cand, (blk, c)
            segs.append((c, cand[0], blk["q0"] - tiles[cand[0]]["q0"]))
        blk["segs"] = segs
        for r in rows:
            for c in _chunks_of_row(r):
                assert (r, c) not in seen
                seen.add((r, c))
    for r in range(H):
        for c in _chunks_of_row(r):
            assert (r, c) in seen, (r, c)

    # masks per tile-pair (0/1), deduped: [128 keys, 2, 512]
    starts = np.minimum(np.maximum(np.arange(H) - KER // 2, 0), H - KER)
    valid = (np.arange(H)[None, :] >= starts[:, None]) & \
            (np.arange(H)[None, :] < starts[:, None] + KER)   # [q pos, k pos]

    def tile_mask(t):
        ktok = t["c"] * 128 + np.arange(128)
        qtok = t["q0"] + np.arange(t["qw"])
        m = np.zeros((128, 512), np.float32)
        m[:, :t["qw"]] = (valid[qtok[None, :] // 64, ktok[:, None] // 64]
                          & valid[qtok[None, :] % 64, ktok[:, None] % 64])
        return m

    mask_list, mask_ids = [], {}
    pair_mask_id = []
    for pi in range(len(tiles) // 2):
        m = np.stack([tile_mask(tiles[2 * pi]), tile_mask(tiles[2 * pi + 1])],
                     axis=1)          # [128, 2, 512]
        key = m.tobytes()
        if key not in mask_ids:
            mask_ids[key] = len(mask_list)
            mask_list.append(m)
        pair_mask_id.append(mask_ids[key])
    return tiles, blocks, pair_mask_id, np.stack(mask_list)


TILES, BLOCKS, PAIR_MASK_ID, MASKS = _build_plan()
NPM = len(MASKS)

# ---------------------------------------------------------------- device

_NC_CACHE = {}
TRACE = False          # set True (e.g. from test.py) to capture an NTFF profile
LAST_RESULTS = None    # BassKernelResults of the most recent kernel() call


def _build_module():
    nc = bacc.Bacc("TRN2", target_bir_lowering=False, debug=False,
                   num_devices=NCORES)
    xT_d = nc.dram_tensor("xT", [D, S], BF16, kind="ExternalInput")
    wq_d = nc.dram_tensor("wq", [D, 128], BF16, kind="ExternalInput")
    wk_d = nc.dram_tensor("wk", [D, 128], BF16, kind="ExternalInput")
    wv_d = nc.dram_tensor("wv", [D, 128], BF16, kind="ExternalInput")
    wo_d = nc.dram_tensor("wo", [128, 512], BF16, kind="ExternalInput")
    mk_d = nc.dram_tensor("masks", [128, NPM * 1024], BF16, kind="ExternalInput")
    y_d = nc.dram_tensor("y", [S, D], BF16, kind="ExternalOutput")

    with tile.TileContext(nc) as tc, ExitStack() as ctx:
        const = ctx.enter_context(tc.tile_pool(name="const", bufs=1))
        xT_t = const.tile([128, 4, S], BF16, tag="xT")
        xr = xT_d.ap().rearrange("(c p) t -> p c t", p=128)
        for ts in range(8):     # split so projections start early
            sl = slice(ts * 512, (ts + 1) * 512)
            nc.sync.dma_start(out=xT_t[:, :, sl], in_=xr[:, :, sl])
        wq_t = const.tile([128, 4, 128], BF16, tag="wq")
        nc.sync.dma_start(out=wq_t[:], in_=wq_d.ap().rearrange("(c p) m -> p c m", p=128))
        wk_t = const.tile([128, 4, 128], BF16, tag="wk")
        nc.sync.dma_start(out=wk_t[:], in_=wk_d.ap().rearrange("(c p) m -> p c m", p=128))
        wv_t = const.tile([128, 4, 128], BF16, tag="wv")
        nc.sync.dma_start(out=wv_t[:], in_=wv_d.ap().rearrange("(c p) m -> p c m", p=128))
        wo_t = const.tile([128, 512], BF16, tag="wo")
        nc.sync.dma_start(out=wo_t[:], in_=wo_d[:, :])
        mk_t = const.tile([128, NPM * 1024], BF16, tag="mk")
        nc.sync.dma_start(out=mk_t[:], in_=mk_d[:, :])

        qT = const.tile([128, S], BF16, tag="qT")      # [2 heads x 64e, tok]
        kT = const.tile([128, S], BF16, tag="kT")
        # V: [tok_in_chunk, chunk, 130]: cols 0:64 u0-e, 64 ones, 65:129 u1-e, 129 ones
        V = const.tile([128, 32, 130], BF16, tag="V")
        nc.gpsimd.memset(V[:], 1.0)
        ident = const.tile([128, 128], BF16, tag="ident")
        make_identity(nc, ident[:])

        # ---- phase 1: projections
        # q/k: dc-outer waves of 4 so the stationary w chunk is reused
        # across 4 matmuls (LDWEIGHTS amortized)
        with tc.tile_pool(name="pps", bufs=4, space="PSUM") as pps:
            for w_t, dst in ((wq_t, qT), (wk_t, kT)):
                for wave in range(2):
                    accs = [pps.tile([128, 512], F32, tag="acc",
                                     name=f"acc_{id(w_t)}_{wave}_{i}")
                            for i in range(4)]
                    for dc in range(4):
                        for i in range(4):
                            nb = wave * 4 + i
                            nc.tensor.matmul(accs[i][:], w_t[:, dc, :],
                                             xT_t[:, dc, nb * 512:(nb + 1) * 512],
                                             start=(dc == 0), stop=(dc == 3))
                    for i in range(4):
                        nb = wave * 4 + i
                        nc.vector.tensor_copy(dst[:, nb * 512:(nb + 1) * 512],
                                              accs[i][:])
            # V in [token, channel] layout directly: xT chunk stationary
            for vb in range(8):
                acc = pps.tile([128, 4, 128], F32, tag="vacc")
                for t4 in range(4):
                    tok0 = (vb * 4 + t4) * 128
                    for dc in range(4):
                        nc.tensor.matmul(acc[:, t4, :],
                                         xT_t[:, dc, tok0:tok0 + 128],
                                         wv_t[:, dc, :],
                                         start=(dc == 0), stop=(dc == 3))
                nc.vector.tensor_copy(V[:, vb * 4:(vb + 1) * 4, 0:64],
                                      acc[:, :, 0:64])
                nc.vector.tensor_copy(V[:, vb * 4:(vb + 1) * 4, 65:129],
                                      acc[:, :, 64:128])

        # ---- phase 2: attention + per-block output projection
        with tc.tile_pool(name="scp", bufs=2, space="PSUM") as scp, \
             tc.tile_pool(name="pvp", bufs=2, space="PSUM") as pvp, \
             tc.tile_pool(name="typ", bufs=1, space="PSUM") as typ, \
             tc.tile_pool(name="prp", bufs=6) as prp, \
             tc.tile_pool(name="aop", bufs=3) as aop, \
             tc.tile_pool(name="atp", bufs=3) as atp, \
             tc.tile_pool(name="rcp", bufs=3) as rcp, \
             tc.tile_pool(name="ysp", bufs=3) as ysp:
            emitted = {}
            alt = [0]

            def ensure_pair(u, pi):
                if (u, pi) in emitted:
                    return
                ue = slice(u * 64, u * 64 + 64)
                sc = scp.tile([128, 1024], F32, tag="sc")
                for s in (0, 1):
                    t = TILES[2 * pi + s]
                    qw, c = t["qw"], t["c"]
                    nc.tensor.matmul(sc[:, s * 512:s * 512 + qw],
                                     kT[ue, c * 128:(c + 1) * 128],
                                     qT[ue, t["q0"]:t["q0"] + qw],
                                     start=True, stop=True)
                pr = prp.tile([128, 1024], BF16, tag="pr")
                nc.scalar.activation(pr[:], sc[:],
                                     mybir.ActivationFunctionType.Exp,
                                     scale=SCALE)
                mid = PAIR_MASK_ID[pi]
                nc.vector.tensor_mul(pr[:], pr[:],
                                     mk_t[:, mid * 1024:(mid + 1) * 1024])
                emitted[(u, pi)] = pr

            for blk in BLOCKS:
                qw, q0 = blk["qw"], blk["q0"]
                for u in (0, 1):
                    for c, ti, off in blk["segs"]:
                        ensure_pair(u, ti // 2)
                pv = pvp.tile([128, 512], F32, tag="pv")
                nseg = len(blk["segs"])
                # all u0 matmuls strictly before u1: the u1 group's start=True
                # clears the whole bank's has_written bits
                for u in (0, 1):
                    u0c = 0 if u == 0 else U1
                    for si, (c, ti, off) in enumerate(blk["segs"]):
                        pr = emitted[(u, ti // 2)]
                        po = (ti % 2) * 512 + off
                        nc.tensor.matmul(pv[:qw, u0c:u0c + 65],
                                         pr[:, po:po + qw],
                                         V[:, c, u * 65:u * 65 + 65],
                                         start=(si == 0), stop=(si == nseg - 1))
                rc = rcp.tile([128, 2], F32, tag="rc")
                nc.vector.reciprocal(rc[:qw, 0:1], pv[:qw, 64:65])
                nc.vector.reciprocal(rc[:qw, 1:2], pv[:qw, U1 + 64:U1 + 65])
                ao = aop.tile([128, 128], BF16, tag="ao")
                nc.vector.tensor_scalar_mul(ao[:qw, 0:64], pv[:qw, 0:64],
                                            rc[:qw, 0:1])
                nc.vector.tensor_scalar_mul(ao[:qw, 64:128], pv[:qw, U1:U1 + 64],
                                            rc[:qw, 1:2])
                tr = typ.tile([128, 1024], BF16, tag="tr")
                nc.tensor.transpose(tr[:, 0:qw], ao[:qw, :], ident[0:qw, 0:qw])
                at = atp.tile([128, 128], BF16, tag="at")
                nc.vector.tensor_copy(at[:, 0:qw], tr[:, 0:qw])
                yo = typ.tile([128, 512], F32, tag="yo")
                nc.tensor.matmul(yo[:qw, :], at[:, 0:qw], wo_t[:],
                                 start=True, stop=True)
                ys = ysp.tile([128, 512], BF16, tag="ys")
                if alt[0] % 2 == 0:
                    nc.vector.tensor_copy(ys[:qw, :], yo[:qw, :])
                else:
                    nc.scalar.activation(ys[:qw, :], yo[:qw, :],
                                         mybir.ActivationFunctionType.Copy)
                alt[0] += 1
                nc.sync.dma_start(out=y_d[q0:q0 + qw, :], in_=ys[:qw, :])
    nc.compile()
    return nc


def _get_module():
    if "nc" not in _NC_CACHE:
        _NC_CACHE["nc"] = _build_module()
    return _NC_CACHE["nc"]


# ---------------------------------------------------------------- host

def kernel(x, w_qkv, w_out):
    x = np.asarray(x, np.float32)
    w_qkv = np.asarray(w_qkv, np.float32)
    w_out = np.asarray(w_out, np.float32)
    nc = _get_module()

    bf = ml_dtypes.bfloat16
    # [NPM, 128, 2, 512] -> [128, NPM*1024]: key-partition major, pairs flat
    masks_bf16 = np.ascontiguousarray(
        MASKS.transpose(1, 0, 2, 3).reshape(128, NPM * 1024)).astype(bf)
    xT = [np.ascontiguousarray(x[b].reshape(S, D).T).astype(bf) for b in range(B)]
    w_outT = np.ascontiguousarray(w_out.T).astype(bf)
    w_qkv = w_qkv.astype(bf)

    in_maps = []
    for c in range(NCORES):
        b, h0 = c // 4, 2 * (c % 4)
        f = h0 * 64
        in_maps.append({
            "xT": xT[b],
            "wq": np.ascontiguousarray(w_qkv[f:f + 128].T),
            "wk": np.ascontiguousarray(w_qkv[512 + f:512 + f + 128].T),
            "wv": np.ascontiguousarray(w_qkv[1024 + f:1024 + f + 128].T),
            "wo": w_outT[f:f + 128],
            "masks": masks_bf16,
        })
    res = run_bass_kernel_spmd(nc, in_maps, list(range(NCORES)), trace=TRACE)
    global LAST_RESULTS
    LAST_RESULTS = res
    y = np.zeros((B, S, D), np.float32)
    for c in range(NCORES):
        y[c // 4] += res.results[c]["y"].astype(np.float32)
    return y.reshape(B, H, W, D)
